# revision 47
# baseline (speedup 1.0000x reference)
"""Trainium2 Bass kernel for nn_AxialBlock (3-axis axial attention sum).

Problem (hardcoded): x (B=4, C=512, T=16, H=32, W=32) fp32, three axial
MHA blocks (attend along W, H, T; n_head=8, d=64) each with their own
QKVO projections; outputs summed. Output (B, C, T, H, W) fp32.

Sharding: 8 cores = (batch b in 0..3) x (half j in 0..1). Every pass is
fully local at 1.0x cost (no collectives, no recompute):
  - w-pass / t-pass: core (b, j) owns the H-half 16j..16j+16 of sample b
    (w-rows and t-fibers lie inside it). Their sum accumulates in an
    SBUF bf16 buffer and is flushed once to output `y`.
  - h-pass: core (b, j) instead computes the T-half 8j..8j+8 of sample b
    (h-rows lie inside a fixed t). Output goes to a second DRAM output
    `y_h`; the HOST adds the two (the (t-half, h-half) ownership grids
    differ, so summing on-device would need a pair exchange).

On-device layout trick: x is channels-first, i.e. already "x^T" (C on
partitions) which is what the PE wants for the QKV projections. The host
pre-permutes x into three token orders (w-fastest / t-fastest / h-fastest)
so that each axial attention acts on 32 consecutive tokens ("rows").

Per 512-token tile (16 rows x 32 tokens):
  q^T (feat-partition) and v (token-partition) projections in bf16; k is
  evacuated parity-split into persistent pre-zeroed "kz" buffers (one head
  per 64 d-rows, rest zero) so attention scores can contract over all 128
  partitions - the PE array tiling positions with BOTH row!=0 and col!=0
  hard-crash the device (NRT_EXEC_UNIT_UNRECOVERABLE), so only (0, col) /
  (row, 0) tiles are usable. Scores: one (K=128, M=32, N=64) matmul per
  (chunk, row) computing both heads of the chunk at col-tile (0, 32j).
  Softmax is batched per 2 row groups with one op per step: exp on
  ScalarE, per-block reduce + reciprocal on VectorE, and the broadcast
  normalize on GpSimd (measured 4x faster there than on VectorE; note
  GpSimd cannot read PSUM, so psum evacuations stay on ScalarE/VectorE).
  The t-pass cross-fiber mask is a rank-2 matmul (-60 additive)
  accumulated under the scores before exp (seq len 16: two t-fibers
  packed per 32-token row).
  A -> A^T via the full-width DVE 32x32 block transpose, then per-row
  contiguous (32, 512) DVE copies form a block-diagonal A^T ("abd") in
  persistent zeroed double buffers; o^T = V^T @ abd lands feat-partition
  directly as one (K=128, M=64, N=128) matmul per (chunk, head).

Scheduling: projection psums accumulate in 2-bank (128, 2, 512) groups on
tag "ps" (2 bufs); o^T/out-proj psums share a single-buffer 2-bank "tail"
tag. Emission is SOFTWARE-PIPELINED (see _make_stages and the driver in
build_program): per-engine queues execute strictly in emission order, so
tile i's tail stages are emitted interleaved between tile i+1's
projection/score groups, and the final out-proj group is skewed one
further iteration - the PE queue then always holds runnable work during
tail-psum evac round-trips.
"""

import contextlib

import ml_dtypes
import numpy as np

import concourse.bass as bass
import concourse.tile as tile
from concourse import bacc, mybir
from concourse.bass_utils import run_bass_kernel_spmd

BF16 = mybir.dt.bfloat16
FP32 = mybir.dt.float32
BF16_NP = np.dtype(ml_dtypes.bfloat16)

B, C, T, H, W = 4, 512, 16, 32, 32
NH, D = 8, 64
HL = H // 2              # per-core H slice
N_CORES = 8
TOK_LOCAL = T * HL * W   # 8192 tokens owned per core
TOK_FULL = T * H * W     # 16384 tokens in a batch sample
TILE = 512               # tokens per on-chip tile
NCH = C // 128           # 4 partition chunks of the feature dim

# dev knob: cap tiles per pass (None = full problem). Truncated builds are
# only for fast AP/scheduling smoke tests - output is wrong when set.
NTILES_CAP = None
# dev knob: repeat the whole workload K times (for overhead attribution:
# fixed per-exec overhead = 2*T(1) - T(2)). Output stays correct for any
# REPS: each rep's w-pass rewrites y_sb before the t-pass adds, and the
# y/y_h writes are idempotent.
REPS = 1
# dev knob: ablations for HW time attribution (output wrong when set):
#   "attn"    - skip S matmuls, softmax and O matmuls (out-proj reads v)
#   "softmax" - keep S and O matmuls, skip the softmax/transpose chain
ABLATE = None


def _make_stages(tc, pools, axis, x_ap, w_aps, y_ap, tml_sb, tmr_sb,
                 kz_tiles, abd_tiles, y_sb, it):
    """Stage thunks for one 512-token tile of one pass.

    The driver emits stages of consecutive tiles INTERLEAVED (software
    pipelining): per-engine queues execute in emission order, so each
    tile's attention-tail psum turnarounds (single-buffer "tail" tag) must
    have the NEXT tile's projection groups queued between them on PE, or
    the PE idles through every tail evac round-trip.

    Stages: a1 (x load + q proj), a2 (k proj -> kz), a3 (v proj),
    b0/b1 (scores + softmax per 2-rowgroup), c1/c2 (o^T per chunk pair),
    c3/c4 (out-projection + per-axis evacuation).
    axis: 'w' write y_sb | 't' strided DVE add into y_sb | 'h' DMA to y_h.
    """
    nc = tc.nc
    wq_sb, wk_sb, wv_sb, wo_sb = w_aps
    (xt_pool, qk_pool, v_pool, a_pool, sm_pool,
     ot_pool, y_pool, ps_pool, sps_pool) = pools
    x3 = x_ap.rearrange("(kc p) n -> p kc n", p=128)
    otw = TILE
    qm = 32                          # query tokens per 32-token row
    GW = NH * 32                     # 256 free columns per row group
    st = {"abd_by_g": {}}

    def a1():
        # x^T tile load + q^T projection (feat-partition bf16). Projection
        # psums use 2-bank (128, 2, 512) groups: two output chunks per psum
        # tile, one wide evac copy.
        xt = xt_pool.tile([128, NCH, TILE], BF16)
        for kg in range(2):
            nc.sync.dma_start(
                xt[:, 2 * kg:2 * kg + 2, :],
                x3[:, 2 * kg:2 * kg + 2, it * TILE:(it + 1) * TILE],
            )
        st["xt"] = xt
        q_sb = qk_pool.tile([128, NCH, TILE], BF16, tag="q")
        for mg in range(2):
            ps = ps_pool.tile([128, 2, TILE], FP32, tag="ps", bufs=2)
            for mc2 in range(2):
                mc = 2 * mg + mc2
                for kc in range(NCH):
                    nc.tensor.matmul(
                        ps[:, mc2, :],
                        lhsT=wq_sb[:, kc, 128 * mc:128 * (mc + 1)],
                        rhs=xt[:, kc, :],
                        start=(kc == 0), stop=(kc == NCH - 1),
                    )
            nc.scalar.copy(q_sb[:, 2 * mg:2 * mg + 2, :], ps[:])
        st["q"] = q_sb

    def a2():
        # k^T projection, evacuated parity-split into the persistent
        # pre-zeroed kz buffers (see module docstring)
        xt = st["xt"]
        kz_sb = kz_tiles[tc._kz_flip]
        tc._kz_flip ^= 1
        for mg in range(2):
            ps = ps_pool.tile([128, 2, TILE], FP32, tag="ps", bufs=2)
            for mc2 in range(2):
                mc = 2 * mg + mc2
                for kc in range(NCH):
                    nc.tensor.matmul(
                        ps[:, mc2, :],
                        lhsT=wk_sb[:, kc, 128 * mc:128 * (mc + 1)],
                        rhs=xt[:, kc, :],
                        start=(kc == 0), stop=(kc == NCH - 1),
                    )
            ms = slice(2 * mg, 2 * mg + 2)
            if mg == 0:
                nc.scalar.copy(kz_sb[0:64, 0, ms, :], ps[0:64, :, :])
                nc.scalar.copy(kz_sb[64:128, 1, ms, :], ps[64:128, :, :])
            else:
                nc.vector.tensor_copy(kz_sb[0:64, 0, ms, :], ps[0:64, :, :])
                nc.vector.tensor_copy(kz_sb[64:128, 1, ms, :], ps[64:128, :, :])
        st["kz"] = kz_sb

    def a3():
        # v projection, token-partition: free = (token block ts, feature)
        xt = st["xt"]
        v_sb = v_pool.tile([128, NCH, C], BF16)
        for tg in range(2):
            ps = ps_pool.tile([128, 2, TILE], FP32, tag="ps", bufs=2)
            for ts2 in range(2):
                ts = 2 * tg + ts2
                for kc in range(NCH):
                    nc.tensor.matmul(
                        ps[:, ts2, :],
                        lhsT=xt[:, kc, 128 * ts:128 * (ts + 1)],
                        rhs=wv_sb[:, kc, :],
                        start=(kc == 0), stop=(kc == NCH - 1),
                    )
            if tg == 0:
                nc.scalar.copy(v_sb[:, 0:2, :], ps[:])
            else:
                nc.vector.tensor_copy(v_sb[:, 2:4, :], ps[:])
        st["v"] = v_sb

    def b(gg):
        # ---- scores + softmax for one 2-rowgroup: S psum (128, 512) = one
        # bank; free = (g%2)*256 + head-slot*32 + kpos. One matmul per
        # (chunk, row) computes BOTH heads of the chunk: the moving operand
        # stacks kz[par=0] and kz[par=1] columns (N=64).
        abd_by_g = st["abd_by_g"]
        if ABLATE == "attn":
            return
        q_sb, kz_sb = st["q"], st["kz"]
        sps = sps_pool.tile([128, 2 * GW], FP32)
        # Base matmul written FIRST with start=True over the full width;
        # the S matmuls then accumulate onto it.
        #   t: rank-2 additive cross-fiber mask (-60 off-fiber, so exp
        #      kills those entries and the reduce needs no mask)
        #   w/h: none needed - S matmuls run standalone (start=True)
        base = axis == "t"
        if base:
            nc.tensor.matmul(
                sps[:], lhsT=tml_sb[:], rhs=tmr_sb[:],
                start=True, stop=False, skip_group_check=True,
            )
        nmm = 32
        i_mm = 0
        for gh in range(2):
            g = 2 * gg + gh
            for c in range(NCH):
                for j in range(4):
                    qcol = (g * 4 + j) * qm
                    i_mm += 1
                    nc.tensor.matmul(
                        sps[32 * j:32 * j + qm,
                            gh * GW + 2 * c * 32:gh * GW + (2 * c + 2) * 32],
                        lhsT=q_sb[:, c, qcol:qcol + qm],
                        rhs=kz_sb[:, :, c,
                                  (g * 4 + j) * 32:(g * 4 + j) * 32 + 32],
                        tile_position=(0, 32 * j),
                        start=(not base),
                        stop=(base and i_mm == nmm),
                        skip_group_check=True,
                    )
        if ABLATE == "softmax":
            abd_by_g[2 * gg] = abd_tiles[gg % 2]
            abd_by_g[2 * gg + 1] = abd_tiles[gg % 2]
            return
        # ---- softmax along k, one op per step per 2 row groups
        a_sb = a_pool.tile([128, 2 * GW], BF16, tag="a")
        nc.scalar.activation(a_sb[:], sps[:],
                             mybir.ActivationFunctionType.Exp)
        a3 = a_sb[:].rearrange("p (n k) -> p n k", n=2 * NH)
        sums = sm_pool.tile([128, 2 * NH], FP32, tag="sums")
        nc.vector.tensor_reduce(
            sums[:], a3, axis=mybir.AxisListType.X, op=mybir.AluOpType.add
        )
        recip = sm_pool.tile([128, 2 * NH], FP32, tag="recip")
        nc.vector.reciprocal(recip[:], sums[:])
        # normalize on GpSimd (measured ~4x faster than DVE for the
        # broadcast multiply), freeing VectorE for the transposes
        nc.gpsimd.tensor_tensor(
            a3, a3,
            recip[:].unsqueeze(2).broadcast_to((128, 2 * NH, 32)),
            mybir.AluOpType.mult,
        )
        # A -> A^T in place (DVE 32x32 block transpose, full width), then
        # per-row contiguous DVE copies into the block-diagonal a_bd buffer
        # (columns row-major j*512 + gh*256 + head*32 + q). Off-diagonal
        # partitions stay zero from the one-time memset.
        at_sb = a_pool.tile([128, 2 * GW], BF16, tag="at")
        nc.vector.transpose(at_sb[:], a_sb[:])
        abd = abd_tiles[tc._abd_flip]
        tc._abd_flip ^= 1
        for j in range(4):
            nc.vector.tensor_copy(
                abd[32 * j:32 * (j + 1), 512 * j:512 * (j + 1)],
                at_sb[32 * j:32 * (j + 1), :],
            )
        abd_by_g[2 * gg] = abd
        abd_by_g[2 * gg + 1] = abd

    def c12(cg):
        # ---- o^T = V^T A_bd for one chunk pair, into the 1-buf "tail"
        # psum tag; evacuate to bf16 SBUF immediately
        v_sb = st["v"]
        if cg == 0:
            st["ot"] = ot_pool.tile([128, NCH, otw], BF16, name="ot_sb")
        ot_sb = st["ot"]
        if ABLATE == "attn":
            for c2 in range(2):
                c = 2 * cg + c2
                nc.gpsimd.tensor_copy(ot_sb[:, c, :], v_sb[:, c, 0:otw])
            return
        abd_by_g = st["abd_by_g"]
        otp = ps_pool.tile([128, 2, otw], FP32, name="otp", tag="tail",
                           bufs=1)
        for c2 in range(2):
            c = 2 * cg + c2
            for g in range(4):
                gh = g % 2
                abd4 = abd_by_g[g][:].rearrange("p (j x) -> p j x", j=4)
                for p in range(2):
                    s0 = gh * GW + (2 * c + p) * 32
                    nc.tensor.matmul(
                        otp[64 * p:64 * (p + 1), c2,
                            g * 4 * qm:(g + 1) * 4 * qm],
                        lhsT=v_sb[:, g, (2 * c + p) * 64:(2 * c + p + 1) * 64],
                        rhs=abd4[:, :, s0:s0 + qm],
                        tile_position=(0, 64 * p),
                    )
        if cg == 0:
            nc.scalar.copy(ot_sb[:, 0:2, :], otp[:])
        else:
            nc.vector.tensor_copy(ot_sb[:, 2:4, :], otp[:])

    def c34(mg):
        # ---- out-projection + per-axis evacuation (bias added host-side)
        ot_sb = st["ot"]
        yps = ps_pool.tile([128, 2, otw], FP32, name="yps", tag="tail",
                           bufs=1)
        for mc2 in range(2):
            mc = 2 * mg + mc2
            for kc in range(NCH):
                nc.tensor.matmul(
                    yps[:, mc2, :],
                    lhsT=wo_sb[:, kc, 128 * mc:128 * (mc + 1)],
                    rhs=ot_sb[:, kc, :],
                    start=(kc == 0), stop=(kc == NCH - 1),
                )
        ms = slice(2 * mg, 2 * mg + 2)
        if axis == "w":
            # first pass: write into the SBUF accumulator (GpSimd cannot
            # read PSUM, so evacuations stay on Scalar/Vector)
            nc.scalar.copy(y_sb[:, ms, it * TILE:(it + 1) * TILE], yps[:])
        elif axis == "t":
            # tile it covers h-row `it`; psum tokens are (w 32, t 16)
            # t-fastest; accumulator side stays natural (t, hl, w)
            y4 = y_sb[:, ms, :].rearrange(
                "p c (t h w) -> p c t h w", t=T, h=HL, w=W)[:, :, :, it, :]
            yp4 = (yps[:].rearrange("p c (w t) -> p c w t", w=W)
                   .transpose([0, 1, 3, 2]))
            nc.vector.tensor_tensor(y4, y4, yp4, mybir.AluOpType.add)
        else:
            # h-pass: plain per-tile write to y_h; token order (t_half, w, h)
            # matches x_h so the write is contiguous
            y_t = y_pool.tile([128, 2, TILE], BF16, tag="yh")
            nc.scalar.copy(y_t[:], yps[:])
            for mc2 in range(2):
                cs = slice(128 * (2 * mg + mc2), 128 * (2 * mg + mc2 + 1))
                nc.sync.dma_start(
                    y_ap[cs, it * TILE:(it + 1) * TILE], y_t[:, mc2, :]
                )

    return {"a1": a1, "a2": a2, "a3": a3,
            "b0": lambda: b(0), "b1": lambda: b(1),
            "c1": lambda: c12(0), "c2": lambda: c12(1),
            "c3": lambda: c34(0), "c4": lambda: c34(1)}


def build_program():
    """Build + compile the SPMD bass program (same program on all 8 cores)."""
    nc = bacc.Bacc(
        "TRN2", target_bir_lowering=False, debug=False,
        enable_asserts=False, num_devices=N_CORES,
    )

    def din(name, shape, dt=BF16):
        return nc.dram_tensor(name, shape, dt, kind="ExternalInput").ap()

    x_w = din("x_w", (C, TOK_LOCAL))
    x_t = din("x_t", (C, TOK_LOCAL))
    x_h = din("x_h", (C, TOK_LOCAL))
    w_in = {}
    for ax in ("w", "t", "h"):
        for nm in ("wq", "wk", "wv", "wo"):
            w_in[f"{nm}_{ax}"] = din(f"{nm}_{ax}", (C, C))
    tml_in = din("tml", (2, 128))
    tmr_in = din("tmr", (2, 512))
    y_ap = nc.dram_tensor("y", (C, TOK_LOCAL), BF16, kind="ExternalOutput").ap()
    yh_ap = nc.dram_tensor("y_h", (C, TOK_LOCAL), BF16,
                           kind="ExternalOutput").ap()

    with tile.TileContext(nc) as tc:
        with contextlib.ExitStack() as ctx:
            xt_pool = ctx.enter_context(tc.tile_pool(name="xt", bufs=4))
            w_pool = ctx.enter_context(tc.tile_pool(name="wts", bufs=2))
            qk_pool = ctx.enter_context(tc.tile_pool(name="qk", bufs=3))
            v_pool = ctx.enter_context(tc.tile_pool(name="v", bufs=3))
            a_pool = ctx.enter_context(tc.tile_pool(name="a", bufs=4))
            sm_pool = ctx.enter_context(tc.tile_pool(name="sm", bufs=3))
            ot_pool = ctx.enter_context(tc.tile_pool(name="ot", bufs=2))
            y_pool = ctx.enter_context(tc.tile_pool(name="y", bufs=3))
            ps_pool = ctx.enter_context(tc.tile_pool(name="ps", bufs=2, space="PSUM"))
            sps_pool = ctx.enter_context(tc.tile_pool(name="sps", bufs=2, space="PSUM"))
            const_pool = ctx.enter_context(tc.tile_pool(name="const", bufs=1))

            # constants
            tml_sb = const_pool.tile([2, 128], BF16)
            nc.sync.dma_start(tml_sb[:], tml_in[:])
            tmr_sb = const_pool.tile([2, 512], BF16)
            nc.sync.dma_start(tmr_sb[:], tmr_in[:])

            # persistent bf16 accumulator for the w+t output sum
            y_sb = const_pool.tile([128, NCH, TOK_LOCAL], BF16, name="yacc")

            # persistent block-diagonal A^T buffers (double-buffered per
            # 2-rowgroup softmax) and parity-split k buffers, zeroed once
            abd_tiles = []
            for i in range(2):
                t = const_pool.tile([128, 4 * 512], BF16, name=f"abd{i}")
                nc.gpsimd.memset(t[:], 0.0)
                abd_tiles.append(t)
            tc._abd_flip = 0
            kz_tiles = []
            for i in range(2):
                t = const_pool.tile([128, 2, NCH, TILE], BF16, name=f"kz{i}")
                nc.gpsimd.memset(t[:], 0.0)
                kz_tiles.append(t)
            tc._kz_flip = 0

            pools = (xt_pool, qk_pool, v_pool, a_pool, sm_pool,
                     ot_pool, y_pool, ps_pool, sps_pool)

            ntiles = TOK_LOCAL // TILE
            if NTILES_CAP is not None:
                ntiles = min(ntiles, NTILES_CAP)
            flat = []
            for _rep in range(REPS):
                for unit in (("w", x_w, y_ap), ("t", x_t, y_ap),
                             ("h", x_h, yh_ap)):
                    for it in range(ntiles):
                        flat.append((*unit, it))

            # Software-pipelined emission: the previous tile's attention
            # tail (c-stages, single-buffer tail psum) is interleaved
            # between this tile's projection/score groups, so the PE queue
            # always has work during tail psum evac round-trips.
            w_cache = {}
            prev = prev2 = None   # tiles at skew -1 and -2
            for i in range(len(flat) + 2):
                stg = None
                if i < len(flat):
                    ax, x_ap, out_ap, it = flat[i]
                    if it == 0:
                        w_aps = []
                        for nm in ("wq", "wk", "wv", "wo"):
                            wt = w_pool.tile([128, NCH, C], BF16, tag=nm,
                                             name=nm)
                            nc.sync.dma_start(
                                wt[:],
                                w_in[f"{nm}_{ax}"].rearrange(
                                    "(kc p) n -> p kc n", p=128),
                            )
                            w_aps.append(wt)
                        w_cache[ax] = w_aps
                    stg = _make_stages(tc, pools, ax, x_ap, w_cache[ax],
                                       out_ap, tml_sb, tmr_sb, kz_tiles,
                                       abd_tiles, y_sb, it)
                    stg["_flush"] = (ax == "t" and it == ntiles - 1)
                if stg:
                    stg["a1"]()
                if prev2:
                    # the last out-proj group of the tile at skew -2: its
                    # tail-psum wait is covered by a1 of the current tile
                    prev2["c4"]()
                    if prev2["_flush"]:
                        # flush the w+t accumulator (overlaps the h-pass)
                        for mc in range(NCH):
                            nc.sync.dma_start(
                                y_ap[128 * mc:128 * (mc + 1), :],
                                y_sb[:, mc, :],
                            )
                if stg:
                    stg["a2"]()
                if prev:
                    prev["c1"]()
                if stg:
                    stg["a3"]()
                if prev:
                    prev["c2"]()
                if stg:
                    stg["b0"]()
                if prev:
                    prev["c3"]()
                if stg:
                    stg["b1"]()
                prev2, prev = prev, stg

    nc.compile()
    return nc


_PROGRAM = None


def _get_program():
    global _PROGRAM
    if _PROGRAM is None:
        _PROGRAM = build_program()
    return _PROGRAM


def make_in_maps(inputs):
    """Host-side shard + layout prep: per-core input dicts."""
    x = np.asarray(inputs["x"], np.float32)          # (B, C, T, H, W)
    scale = 1.0 / np.sqrt(D)

    weights = {}
    for ax in ("w", "h", "t"):
        for nm in ("wq", "wk", "wv", "wo"):
            wm = np.asarray(inputs[f"{nm}_{ax}"], np.float32)
            if nm == "wq":
                wm = wm * scale
            # lhsT layout: (C_in, C_out) = W.T
            weights[f"{nm}_{ax}"] = np.ascontiguousarray(wm.T).astype(BF16_NP)
    _BIAS[0] = (np.asarray(inputs["bo_w"], np.float32)
                + np.asarray(inputs["bo_t"], np.float32)
                + np.asarray(inputs["bo_h"], np.float32))

    # t-pass cross-fiber 0/1 mask: partitions = 4 row-blocks x 32 qpos,
    # free = 32 kpos; two 16-long t-fibers per 32-token row.
    # rank-2 additive cross-fiber mask for the t-pass:
    # S += tml.T @ tmr with tml one-hot on the query fiber and tmr = -60 on
    # cross-fiber key columns
    p = np.arange(128) % 32
    tml = np.stack([(p // 16) == e for e in range(2)]).astype(BF16_NP)
    f = np.arange(512) % 32
    tmr = np.stack([np.where((f // 16) != e, -60.0, 0.0) for e in range(2)]
                   ).astype(BF16_NP)

    in_maps = []
    for core in range(N_CORES):
        b, j = divmod(core, 2)
        xb = x[b]                                    # (C, T, H, W)
        xw = xb[:, :, 16 * j:16 * (j + 1), :]        # (C, T, HL, W) w-fastest
        xt = np.transpose(xw, (0, 2, 3, 1))          # (C, HL, W, T) t-fastest
        # h-pass: this core's T-half of the full-H sample, h-fastest
        xh = np.transpose(xb[:, 8 * j:8 * (j + 1), :, :], (0, 1, 3, 2))
        m = {
            "x_w": np.ascontiguousarray(xw).reshape(C, TOK_LOCAL).astype(BF16_NP),
            "x_t": np.ascontiguousarray(xt).reshape(C, TOK_LOCAL).astype(BF16_NP),
            "x_h": np.ascontiguousarray(xh).reshape(C, TOK_LOCAL).astype(BF16_NP),
            "tml": tml, "tmr": tmr,
        }
        m.update(weights)
        in_maps.append(m)
    return in_maps


_BIAS = [None]   # summed output bias (C,), applied host-side in assemble


def assemble_output(results):
    """Host gather: y (w+t, H-half shard) + y_h (h-pass, T-half shard)."""
    out = np.empty((B, C, T, H, W), np.float32)
    for core in range(N_CORES):
        b, j = divmod(core, 2)
        y = np.asarray(results[core]["y"]).astype(np.float32)
        out[b, :, :, 16 * j:16 * (j + 1), :] = y.reshape(C, T, HL, W)
    for core in range(N_CORES):
        b, j = divmod(core, 2)
        yh = (np.asarray(results[core]["y_h"]).astype(np.float32)
              .reshape(C, 8, W, H))
        out[b, :, 8 * j:8 * (j + 1), :, :] += yh.transpose(0, 1, 3, 2)
    if _BIAS[0] is not None and _BIAS[0].any():
        out += _BIAS[0].reshape(1, C, 1, 1, 1)
    return out


_RUNNER = None


def _get_runner():
    """Build the sharded PJRT callable once; reuse across kernel() calls."""
    global _RUNNER
    if _RUNNER is not None:
        return _RUNNER
    import jax
    from jax.sharding import Mesh, PartitionSpec
    from jax.experimental.shard_map import shard_map
    from concourse import bass2jax

    nc = _get_program()
    bass2jax.install_neuronx_cc_hook()
    partition_name = (nc.partition_id_tensor.name
                      if nc.partition_id_tensor else None)
    in_names, out_names, out_avals, zero_outs = [], [], [], []
    for alloc in nc.m.functions[0].allocations:
        if not isinstance(alloc, mybir.MemoryLocationSet):
            continue
        name = alloc.memorylocations[0].name
        if alloc.kind == "ExternalInput":
            if name != partition_name:
                in_names.append(name)
        elif alloc.kind == "ExternalOutput":
            out_names.append(name)
            shape = tuple(alloc.tensor_shape)
            dtype = mybir.dt.np(alloc.dtype)
            out_avals.append(jax.core.ShapedArray(shape, dtype))
            zero_outs.append(np.zeros((N_CORES * shape[0], *shape[1:]), dtype))
    n_params = len(in_names)
    all_in_names = list(in_names) + out_names
    if partition_name is not None:
        all_in_names.append(partition_name)

    def _body(*args):
        operands = list(args)
        if partition_name is not None:
            operands.append(bass2jax.partition_id_tensor())
        return tuple(bass2jax._bass_exec_p.bind(
            *operands,
            out_avals=tuple(out_avals),
            in_names=tuple(all_in_names),
            out_names=tuple(out_names),
            lowering_input_output_aliases=(),
            sim_require_finite=True,
            sim_require_nnan=True,
            nc=nc,
        ))

    devices = jax.devices()[:N_CORES]
    mesh = Mesh(np.asarray(devices), ("core",))
    in_specs = (PartitionSpec("core"),) * (n_params + len(out_names))
    out_specs = (PartitionSpec("core"),) * len(out_names)
    fn = jax.jit(shard_map(_body, mesh=mesh, in_specs=in_specs,
                           out_specs=out_specs, check_rep=False))

    def run(in_maps):
        concat_in = [
            np.concatenate([np.asarray(in_maps[c][nm]) for c in range(N_CORES)],
                           axis=0)
            for nm in in_names
        ]
        outs = fn(*concat_in, *zero_outs)
        return [
            {nm: np.asarray(outs[i]).reshape(N_CORES, *out_avals[i].shape)[c]
             for i, nm in enumerate(out_names)}
            for c in range(N_CORES)
        ]

    _RUNNER = run
    return run


def kernel(**inputs) -> np.ndarray:
    run = _get_runner()
    in_maps = make_in_maps(inputs)
    return assemble_output(run(in_maps))



# revision 48
# speedup vs baseline: 1.3999x; 1.3999x over previous
"""Trainium2 Bass kernel for nn_AxialBlock (3-axis axial attention sum).

Problem (hardcoded): x (B=4, C=512, T=16, H=32, W=32) fp32, three axial
MHA blocks (attend along W, H, T; n_head=8, d=64) each with their own
QKVO projections; outputs summed. Output (B, C, T, H, W) fp32.

Sharding: 8 cores = (batch b in 0..3) x (half j in 0..1). Every pass is
fully local at 1.0x cost (no collectives, no recompute):
  - w-pass / t-pass: core (b, j) owns the H-half 16j..16j+16 of sample b
    (w-rows and t-fibers lie inside it). Their sum accumulates in an
    SBUF bf16 buffer and is flushed once to output `y`.
  - h-pass: core (b, j) instead computes the T-half 8j..8j+8 of sample b
    (h-rows lie inside a fixed t). Output goes to a second DRAM output
    `y_h`; the HOST adds the two (the (t-half, h-half) ownership grids
    differ, so summing on-device would need a pair exchange).

On-device layout trick: x is channels-first, i.e. already "x^T" (C on
partitions) which is what the PE wants for the QKV projections. The host
pre-permutes x into three token orders (w-fastest / t-fastest / h-fastest)
so that each axial attention acts on 32 consecutive tokens ("rows").

Per 512-token tile (16 rows x 32 tokens):
  q^T (feat-partition) and v (token-partition) projections in bf16; k is
  evacuated parity-split into persistent pre-zeroed "kz" buffers (one head
  per 64 d-rows, rest zero) so attention scores can contract over all 128
  partitions - the PE array tiling positions with BOTH row!=0 and col!=0
  hard-crash the device (NRT_EXEC_UNIT_UNRECOVERABLE), so only (0, col) /
  (row, 0) tiles are usable. Scores: one (K=128, M=32, N=64) matmul per
  (chunk, row) computing both heads of the chunk at col-tile (0, 32j).
  Softmax is batched per 2 row groups with one op per step: exp on
  ScalarE, per-block reduce + reciprocal on VectorE, and the broadcast
  normalize on GpSimd (measured 4x faster there than on VectorE; note
  GpSimd cannot read PSUM, so psum evacuations stay on ScalarE/VectorE).
  The t-pass cross-fiber mask is a rank-2 matmul (-60 additive)
  accumulated under the scores before exp (seq len 16: two t-fibers
  packed per 32-token row).
  A -> A^T via the full-width DVE 32x32 block transpose, then per-row
  contiguous (32, 512) DVE copies form a block-diagonal A^T ("abd") in
  persistent zeroed double buffers; o^T = V^T @ abd lands feat-partition
  directly as one (K=128, M=64, N=128) matmul per (chunk, head).

Scheduling: projection psums accumulate in 2-bank (128, 2, 512) groups on
tag "ps" (2 bufs); o^T/out-proj psums share a single-buffer 2-bank "tail"
tag. Emission is SOFTWARE-PIPELINED (see _make_stages and the driver in
build_program): per-engine queues execute strictly in emission order, so
tile i's tail stages are emitted interleaved between tile i+1's
projection/score groups, and the final out-proj group is skewed one
further iteration - the PE queue then always holds runnable work during
tail-psum evac round-trips.
"""

import contextlib

import ml_dtypes
import numpy as np

import concourse.bass as bass
import concourse.tile as tile
from concourse import bacc, mybir
from concourse.bass_utils import run_bass_kernel_spmd

BF16 = mybir.dt.bfloat16
FP32 = mybir.dt.float32
BF16_NP = np.dtype(ml_dtypes.bfloat16)

B, C, T, H, W = 4, 512, 16, 32, 32
NH, D = 8, 64
HL = H // 2              # per-core H slice
N_CORES = 8
TOK_LOCAL = T * HL * W   # 8192 tokens owned per core
TOK_FULL = T * H * W     # 16384 tokens in a batch sample
TILE = 512               # tokens per on-chip tile
NCH = C // 128           # 4 partition chunks of the feature dim

# dev knob: cap tiles per pass (None = full problem). Truncated builds are
# only for fast AP/scheduling smoke tests - output is wrong when set.
NTILES_CAP = None
# dev knob: repeat the whole workload K times (for overhead attribution:
# fixed per-exec overhead = 2*T(1) - T(2)). Output stays correct for any
# REPS: each rep's w-pass rewrites y_sb before the t-pass adds, and the
# y/y_h writes are idempotent.
REPS = 1
# dev knob: ablations for HW time attribution (output wrong when set):
#   "attn"    - skip S matmuls, softmax and O matmuls (out-proj reads v)
#   "softmax" - keep S and O matmuls, skip the softmax/transpose chain
ABLATE = None


def _make_stages(tc, pools, axis, x_ap, w_aps, y_ap, tml_sb, tmr_sb,
                 kz_tiles, abd_tiles, y_sb, it):
    """Stage thunks for one 512-token tile of one pass.

    The driver emits stages of consecutive tiles INTERLEAVED (software
    pipelining): per-engine queues execute in emission order, so each
    tile's attention-tail psum turnarounds (single-buffer "tail" tag) must
    have the NEXT tile's projection groups queued between them on PE, or
    the PE idles through every tail evac round-trip.

    Stages: a1 (x load + q proj), a2 (k proj -> kz), a3 (v proj),
    b0/b1 (scores + softmax per 2-rowgroup), c1/c2 (o^T per chunk pair),
    c3/c4 (out-projection + per-axis evacuation).
    axis: 'w' write y_sb | 't' strided DVE add into y_sb | 'h' DMA to y_h.
    """
    nc = tc.nc
    wq_sb, wk_sb, wv_sb, wo_sb = w_aps
    (xt_pool, qk_pool, v_pool, a_pool, sm_pool,
     ot_pool, y_pool, ps_pool, sps_pool) = pools
    x3 = x_ap.rearrange("(kc p) n -> p kc n", p=128)
    otw = TILE
    qm = 32                          # query tokens per 32-token row
    GW = NH * 32                     # 256 free columns per row group
    st = {"abd_by_g": {}}

    def a1():
        # x^T tile load + q^T projection (feat-partition bf16). Projection
        # psums use 2-bank (128, 2, 512) groups: two output chunks per psum
        # tile, one wide evac copy.
        xt = xt_pool.tile([128, NCH, TILE], BF16)
        for kg in range(2):
            nc.sync.dma_start(
                xt[:, 2 * kg:2 * kg + 2, :],
                x3[:, 2 * kg:2 * kg + 2, it * TILE:(it + 1) * TILE],
            )
        st["xt"] = xt
        q_sb = qk_pool.tile([128, NCH, TILE], BF16, tag="q")
        for mg in range(2):
            ps = ps_pool.tile([128, 2, TILE], FP32, tag="ps", bufs=2)
            for mc2 in range(2):
                mc = 2 * mg + mc2
                for kc in range(NCH):
                    nc.tensor.matmul(
                        ps[:, mc2, :],
                        lhsT=wq_sb[:, kc, 128 * mc:128 * (mc + 1)],
                        rhs=xt[:, kc, :],
                        start=(kc == 0), stop=(kc == NCH - 1),
                    )
            nc.scalar.copy(q_sb[:, 2 * mg:2 * mg + 2, :], ps[:])
        st["q"] = q_sb

    def a2():
        # k^T projection, evacuated parity-split into the persistent
        # pre-zeroed kz buffers (see module docstring)
        xt = st["xt"]
        kz_sb = kz_tiles[tc._kz_flip]
        tc._kz_flip ^= 1
        for mg in range(2):
            ps = ps_pool.tile([128, 2, TILE], FP32, tag="ps", bufs=2)
            for mc2 in range(2):
                mc = 2 * mg + mc2
                for kc in range(NCH):
                    nc.tensor.matmul(
                        ps[:, mc2, :],
                        lhsT=wk_sb[:, kc, 128 * mc:128 * (mc + 1)],
                        rhs=xt[:, kc, :],
                        start=(kc == 0), stop=(kc == NCH - 1),
                    )
            ms = slice(2 * mg, 2 * mg + 2)
            if mg == 0:
                nc.scalar.copy(kz_sb[0:64, 0, ms, :], ps[0:64, :, :])
                nc.scalar.copy(kz_sb[64:128, 1, ms, :], ps[64:128, :, :])
            else:
                nc.vector.tensor_copy(kz_sb[0:64, 0, ms, :], ps[0:64, :, :])
                nc.vector.tensor_copy(kz_sb[64:128, 1, ms, :], ps[64:128, :, :])
        st["kz"] = kz_sb

    def a3():
        # v projection, token-partition: free = (token block ts, feature)
        xt = st["xt"]
        v_sb = v_pool.tile([128, NCH, C], BF16)
        for tg in range(2):
            ps = ps_pool.tile([128, 2, TILE], FP32, tag="ps", bufs=2)
            for ts2 in range(2):
                ts = 2 * tg + ts2
                for kc in range(NCH):
                    nc.tensor.matmul(
                        ps[:, ts2, :],
                        lhsT=xt[:, kc, 128 * ts:128 * (ts + 1)],
                        rhs=wv_sb[:, kc, :],
                        start=(kc == 0), stop=(kc == NCH - 1),
                    )
            if tg == 0:
                nc.scalar.copy(v_sb[:, 0:2, :], ps[:])
            else:
                nc.vector.tensor_copy(v_sb[:, 2:4, :], ps[:])
        st["v"] = v_sb

    def b(gg):
        # ---- scores + softmax for one 2-rowgroup: S psum (128, 512) = one
        # bank; free = (g%2)*256 + head-slot*32 + kpos. One matmul per
        # (chunk, row) computes BOTH heads of the chunk: the moving operand
        # stacks kz[par=0] and kz[par=1] columns (N=64).
        abd_by_g = st["abd_by_g"]
        if ABLATE == "attn":
            return
        q_sb, kz_sb = st["q"], st["kz"]
        sps = sps_pool.tile([128, 2 * GW], FP32)
        # Base matmul written FIRST with start=True over the full width;
        # the S matmuls then accumulate onto it.
        #   t: rank-2 additive cross-fiber mask (-60 off-fiber, so exp
        #      kills those entries and the reduce needs no mask)
        #   w/h: none needed - S matmuls run standalone (start=True)
        base = axis == "t"
        if base:
            nc.tensor.matmul(
                sps[:], lhsT=tml_sb[:], rhs=tmr_sb[:],
                start=True, stop=False, skip_group_check=True,
            )
        # j (= PE column-tile position) is the OUTER loop so consecutive
        # matmuls keep the same tile_position (any per-switch PE
        # reconfiguration cost is paid 4x per rowgroup, not 32x)
        nmm = 32
        i_mm = 0
        for j in range(4):
            for gh in range(2):
                g = 2 * gg + gh
                for c in range(NCH):
                    qcol = (g * 4 + j) * qm
                    i_mm += 1
                    nc.tensor.matmul(
                        sps[32 * j:32 * j + qm,
                            gh * GW + 2 * c * 32:gh * GW + (2 * c + 2) * 32],
                        lhsT=q_sb[:, c, qcol:qcol + qm],
                        rhs=kz_sb[:, :, c,
                                  (g * 4 + j) * 32:(g * 4 + j) * 32 + 32],
                        tile_position=(0, 32 * j),
                        start=(not base),
                        stop=(base and i_mm == nmm),
                        skip_group_check=True,
                    )
        if ABLATE == "softmax":
            abd_by_g[2 * gg] = abd_tiles[gg % 2]
            abd_by_g[2 * gg + 1] = abd_tiles[gg % 2]
            return
        # ---- softmax along k, one op per step per 2 row groups
        a_sb = a_pool.tile([128, 2 * GW], BF16, tag="a")
        nc.scalar.activation(a_sb[:], sps[:],
                             mybir.ActivationFunctionType.Exp)
        a3 = a_sb[:].rearrange("p (n k) -> p n k", n=2 * NH)
        sums = sm_pool.tile([128, 2 * NH], FP32, tag="sums")
        nc.vector.tensor_reduce(
            sums[:], a3, axis=mybir.AxisListType.X, op=mybir.AluOpType.add
        )
        recip = sm_pool.tile([128, 2 * NH], FP32, tag="recip")
        nc.vector.reciprocal(recip[:], sums[:])
        # normalize on GpSimd (measured ~4x faster than DVE for the
        # broadcast multiply), freeing VectorE for the transposes
        nc.gpsimd.tensor_tensor(
            a3, a3,
            recip[:].unsqueeze(2).broadcast_to((128, 2 * NH, 32)),
            mybir.AluOpType.mult,
        )
        # A -> A^T in place (DVE 32x32 block transpose, full width), then
        # per-row contiguous DVE copies into the block-diagonal a_bd buffer
        # (columns row-major j*512 + gh*256 + head*32 + q). Off-diagonal
        # partitions stay zero from the one-time memset.
        at_sb = a_pool.tile([128, 2 * GW], BF16, tag="at")
        nc.vector.transpose(at_sb[:], a_sb[:])
        abd = abd_tiles[tc._abd_flip]
        tc._abd_flip ^= 1
        for j in range(4):
            nc.vector.tensor_copy(
                abd[32 * j:32 * (j + 1), 512 * j:512 * (j + 1)],
                at_sb[32 * j:32 * (j + 1), :],
            )
        abd_by_g[2 * gg] = abd
        abd_by_g[2 * gg + 1] = abd

    def c12(cg):
        # ---- o^T = V^T A_bd for one chunk pair, into the 1-buf "tail"
        # psum tag; evacuate to bf16 SBUF immediately
        v_sb = st["v"]
        if cg == 0:
            st["ot"] = ot_pool.tile([128, NCH, otw], BF16, name="ot_sb")
        ot_sb = st["ot"]
        if ABLATE == "attn":
            for c2 in range(2):
                c = 2 * cg + c2
                nc.gpsimd.tensor_copy(ot_sb[:, c, :], v_sb[:, c, 0:otw])
            return
        abd_by_g = st["abd_by_g"]
        otp = ps_pool.tile([128, 2, otw], FP32, name="otp", tag="tail",
                           bufs=1)
        for c2 in range(2):
            c = 2 * cg + c2
            for p in range(2):
                for g in range(4):
                    gh = g % 2
                    abd4 = abd_by_g[g][:].rearrange("p (j x) -> p j x", j=4)
                    s0 = gh * GW + (2 * c + p) * 32
                    nc.tensor.matmul(
                        otp[64 * p:64 * (p + 1), c2,
                            g * 4 * qm:(g + 1) * 4 * qm],
                        lhsT=v_sb[:, g, (2 * c + p) * 64:(2 * c + p + 1) * 64],
                        rhs=abd4[:, :, s0:s0 + qm],
                        tile_position=(0, 64 * p),
                    )
        if cg == 0:
            nc.scalar.copy(ot_sb[:, 0:2, :], otp[:])
        else:
            nc.vector.tensor_copy(ot_sb[:, 2:4, :], otp[:])

    def c34(mg):
        # ---- out-projection + per-axis evacuation (bias added host-side)
        ot_sb = st["ot"]
        yps = ps_pool.tile([128, 2, otw], FP32, name="yps", tag="tail",
                           bufs=1)
        for mc2 in range(2):
            mc = 2 * mg + mc2
            for kc in range(NCH):
                nc.tensor.matmul(
                    yps[:, mc2, :],
                    lhsT=wo_sb[:, kc, 128 * mc:128 * (mc + 1)],
                    rhs=ot_sb[:, kc, :],
                    start=(kc == 0), stop=(kc == NCH - 1),
                )
        ms = slice(2 * mg, 2 * mg + 2)
        if axis == "w":
            # first pass: write into the SBUF accumulator (GpSimd cannot
            # read PSUM, so evacuations stay on Scalar/Vector)
            nc.scalar.copy(y_sb[:, ms, it * TILE:(it + 1) * TILE], yps[:])
        elif axis == "t":
            # tile it covers h-row `it`; psum tokens are (w 32, t 16)
            # t-fastest; accumulator side stays natural (t, hl, w)
            y4 = y_sb[:, ms, :].rearrange(
                "p c (t h w) -> p c t h w", t=T, h=HL, w=W)[:, :, :, it, :]
            yp4 = (yps[:].rearrange("p c (w t) -> p c w t", w=W)
                   .transpose([0, 1, 3, 2]))
            nc.vector.tensor_tensor(y4, y4, yp4, mybir.AluOpType.add)
        else:
            # h-pass: plain per-tile write to y_h; token order (t_half, w, h)
            # matches x_h so the write is contiguous
            y_t = y_pool.tile([128, 2, TILE], BF16, tag="yh")
            nc.scalar.copy(y_t[:], yps[:])
            for mc2 in range(2):
                cs = slice(128 * (2 * mg + mc2), 128 * (2 * mg + mc2 + 1))
                nc.sync.dma_start(
                    y_ap[cs, it * TILE:(it + 1) * TILE], y_t[:, mc2, :]
                )

    return {"a1": a1, "a2": a2, "a3": a3,
            "b0": lambda: b(0), "b1": lambda: b(1),
            "c1": lambda: c12(0), "c2": lambda: c12(1),
            "c3": lambda: c34(0), "c4": lambda: c34(1)}


def build_program():
    """Build + compile the SPMD bass program (same program on all 8 cores)."""
    nc = bacc.Bacc(
        "TRN2", target_bir_lowering=False, debug=False,
        enable_asserts=False, num_devices=N_CORES,
    )

    def din(name, shape, dt=BF16):
        return nc.dram_tensor(name, shape, dt, kind="ExternalInput").ap()

    x_w = din("x_w", (C, TOK_LOCAL))
    x_t = din("x_t", (C, TOK_LOCAL))
    x_h = din("x_h", (C, TOK_LOCAL))
    w_in = {}
    for ax in ("w", "t", "h"):
        for nm in ("wq", "wk", "wv", "wo"):
            w_in[f"{nm}_{ax}"] = din(f"{nm}_{ax}", (C, C))
    tml_in = din("tml", (2, 128))
    tmr_in = din("tmr", (2, 512))
    y_ap = nc.dram_tensor("y", (C, TOK_LOCAL), BF16, kind="ExternalOutput").ap()
    yh_ap = nc.dram_tensor("y_h", (C, TOK_LOCAL), BF16,
                           kind="ExternalOutput").ap()

    with tile.TileContext(nc) as tc:
        with contextlib.ExitStack() as ctx:
            xt_pool = ctx.enter_context(tc.tile_pool(name="xt", bufs=4))
            w_pool = ctx.enter_context(tc.tile_pool(name="wts", bufs=2))
            qk_pool = ctx.enter_context(tc.tile_pool(name="qk", bufs=3))
            v_pool = ctx.enter_context(tc.tile_pool(name="v", bufs=3))
            a_pool = ctx.enter_context(tc.tile_pool(name="a", bufs=4))
            sm_pool = ctx.enter_context(tc.tile_pool(name="sm", bufs=3))
            ot_pool = ctx.enter_context(tc.tile_pool(name="ot", bufs=2))
            y_pool = ctx.enter_context(tc.tile_pool(name="y", bufs=3))
            ps_pool = ctx.enter_context(tc.tile_pool(name="ps", bufs=2, space="PSUM"))
            sps_pool = ctx.enter_context(tc.tile_pool(name="sps", bufs=2, space="PSUM"))
            const_pool = ctx.enter_context(tc.tile_pool(name="const", bufs=1))

            # constants
            tml_sb = const_pool.tile([2, 128], BF16)
            nc.sync.dma_start(tml_sb[:], tml_in[:])
            tmr_sb = const_pool.tile([2, 512], BF16)
            nc.sync.dma_start(tmr_sb[:], tmr_in[:])

            # persistent bf16 accumulator for the w+t output sum
            y_sb = const_pool.tile([128, NCH, TOK_LOCAL], BF16, name="yacc")

            # persistent block-diagonal A^T buffers (double-buffered per
            # 2-rowgroup softmax) and parity-split k buffers, zeroed once
            abd_tiles = []
            for i in range(2):
                t = const_pool.tile([128, 4 * 512], BF16, name=f"abd{i}")
                nc.gpsimd.memset(t[:], 0.0)
                abd_tiles.append(t)
            tc._abd_flip = 0
            kz_tiles = []
            for i in range(2):
                t = const_pool.tile([128, 2, NCH, TILE], BF16, name=f"kz{i}")
                nc.gpsimd.memset(t[:], 0.0)
                kz_tiles.append(t)
            tc._kz_flip = 0

            pools = (xt_pool, qk_pool, v_pool, a_pool, sm_pool,
                     ot_pool, y_pool, ps_pool, sps_pool)

            ntiles = TOK_LOCAL // TILE
            if NTILES_CAP is not None:
                ntiles = min(ntiles, NTILES_CAP)
            flat = []
            for _rep in range(REPS):
                for unit in (("w", x_w, y_ap), ("t", x_t, y_ap),
                             ("h", x_h, yh_ap)):
                    for it in range(ntiles):
                        flat.append((*unit, it))

            # Software-pipelined emission: the previous tile's attention
            # tail (c-stages, single-buffer tail psum) is interleaved
            # between this tile's projection/score groups, so the PE queue
            # always has work during tail psum evac round-trips.
            w_cache = {}
            prev = prev2 = None   # tiles at skew -1 and -2
            for i in range(len(flat) + 2):
                stg = None
                if i < len(flat):
                    ax, x_ap, out_ap, it = flat[i]
                    if it == 0:
                        w_aps = []
                        for nm in ("wq", "wk", "wv", "wo"):
                            wt = w_pool.tile([128, NCH, C], BF16, tag=nm,
                                             name=nm)
                            nc.sync.dma_start(
                                wt[:],
                                w_in[f"{nm}_{ax}"].rearrange(
                                    "(kc p) n -> p kc n", p=128),
                            )
                            w_aps.append(wt)
                        w_cache[ax] = w_aps
                    stg = _make_stages(tc, pools, ax, x_ap, w_cache[ax],
                                       out_ap, tml_sb, tmr_sb, kz_tiles,
                                       abd_tiles, y_sb, it)
                    stg["_flush"] = (ax == "t" and it == ntiles - 1)
                if stg:
                    stg["a1"]()
                if prev2:
                    # the last out-proj group of the tile at skew -2: its
                    # tail-psum wait is covered by a1 of the current tile
                    prev2["c4"]()
                    if prev2["_flush"]:
                        # flush the w+t accumulator (overlaps the h-pass)
                        for mc in range(NCH):
                            nc.sync.dma_start(
                                y_ap[128 * mc:128 * (mc + 1), :],
                                y_sb[:, mc, :],
                            )
                if stg:
                    stg["a2"]()
                if prev:
                    prev["c1"]()
                if stg:
                    stg["a3"]()
                if prev:
                    prev["c2"]()
                if stg:
                    stg["b0"]()
                if prev:
                    prev["c3"]()
                if stg:
                    stg["b1"]()
                prev2, prev = prev, stg

    nc.compile()
    return nc


_PROGRAM = None


def _get_program():
    global _PROGRAM
    if _PROGRAM is None:
        _PROGRAM = build_program()
    return _PROGRAM


def make_in_maps(inputs):
    """Host-side shard + layout prep: per-core input dicts."""
    x = np.asarray(inputs["x"], np.float32)          # (B, C, T, H, W)
    scale = 1.0 / np.sqrt(D)

    weights = {}
    for ax in ("w", "h", "t"):
        for nm in ("wq", "wk", "wv", "wo"):
            wm = np.asarray(inputs[f"{nm}_{ax}"], np.float32)
            if nm == "wq":
                wm = wm * scale
            # lhsT layout: (C_in, C_out) = W.T
            weights[f"{nm}_{ax}"] = np.ascontiguousarray(wm.T).astype(BF16_NP)
    _BIAS[0] = (np.asarray(inputs["bo_w"], np.float32)
                + np.asarray(inputs["bo_t"], np.float32)
                + np.asarray(inputs["bo_h"], np.float32))

    # t-pass cross-fiber 0/1 mask: partitions = 4 row-blocks x 32 qpos,
    # free = 32 kpos; two 16-long t-fibers per 32-token row.
    # rank-2 additive cross-fiber mask for the t-pass:
    # S += tml.T @ tmr with tml one-hot on the query fiber and tmr = -60 on
    # cross-fiber key columns
    p = np.arange(128) % 32
    tml = np.stack([(p // 16) == e for e in range(2)]).astype(BF16_NP)
    f = np.arange(512) % 32
    tmr = np.stack([np.where((f // 16) != e, -60.0, 0.0) for e in range(2)]
                   ).astype(BF16_NP)

    in_maps = []
    for core in range(N_CORES):
        b, j = divmod(core, 2)
        xb = x[b]                                    # (C, T, H, W)
        xw = xb[:, :, 16 * j:16 * (j + 1), :]        # (C, T, HL, W) w-fastest
        xt = np.transpose(xw, (0, 2, 3, 1))          # (C, HL, W, T) t-fastest
        # h-pass: this core's T-half of the full-H sample, h-fastest
        xh = np.transpose(xb[:, 8 * j:8 * (j + 1), :, :], (0, 1, 3, 2))
        m = {
            "x_w": np.ascontiguousarray(xw).reshape(C, TOK_LOCAL).astype(BF16_NP),
            "x_t": np.ascontiguousarray(xt).reshape(C, TOK_LOCAL).astype(BF16_NP),
            "x_h": np.ascontiguousarray(xh).reshape(C, TOK_LOCAL).astype(BF16_NP),
            "tml": tml, "tmr": tmr,
        }
        m.update(weights)
        in_maps.append(m)
    return in_maps


_BIAS = [None]   # summed output bias (C,), applied host-side in assemble


def assemble_output(results):
    """Host gather: y (w+t, H-half shard) + y_h (h-pass, T-half shard)."""
    out = np.empty((B, C, T, H, W), np.float32)
    for core in range(N_CORES):
        b, j = divmod(core, 2)
        y = np.asarray(results[core]["y"]).astype(np.float32)
        out[b, :, :, 16 * j:16 * (j + 1), :] = y.reshape(C, T, HL, W)
    for core in range(N_CORES):
        b, j = divmod(core, 2)
        yh = (np.asarray(results[core]["y_h"]).astype(np.float32)
              .reshape(C, 8, W, H))
        out[b, :, 8 * j:8 * (j + 1), :, :] += yh.transpose(0, 1, 3, 2)
    if _BIAS[0] is not None and _BIAS[0].any():
        out += _BIAS[0].reshape(1, C, 1, 1, 1)
    return out


_RUNNER = None


def _get_runner():
    """Build the sharded PJRT callable once; reuse across kernel() calls."""
    global _RUNNER
    if _RUNNER is not None:
        return _RUNNER
    import jax
    from jax.sharding import Mesh, PartitionSpec
    from jax.experimental.shard_map import shard_map
    from concourse import bass2jax

    nc = _get_program()
    bass2jax.install_neuronx_cc_hook()
    partition_name = (nc.partition_id_tensor.name
                      if nc.partition_id_tensor else None)
    in_names, out_names, out_avals, zero_outs = [], [], [], []
    for alloc in nc.m.functions[0].allocations:
        if not isinstance(alloc, mybir.MemoryLocationSet):
            continue
        name = alloc.memorylocations[0].name
        if alloc.kind == "ExternalInput":
            if name != partition_name:
                in_names.append(name)
        elif alloc.kind == "ExternalOutput":
            out_names.append(name)
            shape = tuple(alloc.tensor_shape)
            dtype = mybir.dt.np(alloc.dtype)
            out_avals.append(jax.core.ShapedArray(shape, dtype))
            zero_outs.append(np.zeros((N_CORES * shape[0], *shape[1:]), dtype))
    n_params = len(in_names)
    all_in_names = list(in_names) + out_names
    if partition_name is not None:
        all_in_names.append(partition_name)

    def _body(*args):
        operands = list(args)
        if partition_name is not None:
            operands.append(bass2jax.partition_id_tensor())
        return tuple(bass2jax._bass_exec_p.bind(
            *operands,
            out_avals=tuple(out_avals),
            in_names=tuple(all_in_names),
            out_names=tuple(out_names),
            lowering_input_output_aliases=(),
            sim_require_finite=True,
            sim_require_nnan=True,
            nc=nc,
        ))

    devices = jax.devices()[:N_CORES]
    mesh = Mesh(np.asarray(devices), ("core",))
    in_specs = (PartitionSpec("core"),) * (n_params + len(out_names))
    out_specs = (PartitionSpec("core"),) * len(out_names)
    fn = jax.jit(shard_map(_body, mesh=mesh, in_specs=in_specs,
                           out_specs=out_specs, check_rep=False))

    def run(in_maps):
        concat_in = [
            np.concatenate([np.asarray(in_maps[c][nm]) for c in range(N_CORES)],
                           axis=0)
            for nm in in_names
        ]
        outs = fn(*concat_in, *zero_outs)
        return [
            {nm: np.asarray(outs[i]).reshape(N_CORES, *out_avals[i].shape)[c]
             for i, nm in enumerate(out_names)}
            for c in range(N_CORES)
        ]

    _RUNNER = run
    return run


def kernel(**inputs) -> np.ndarray:
    run = _get_runner()
    in_maps = make_in_maps(inputs)
    return assemble_output(run(in_maps))

